# revision 8
# baseline (speedup 1.0000x reference)
"""Multi-head causal attention (GPT-2 style) on 8 TRN2 NeuronCores.

Sharding: core i handles batch i//2 and head-group i%2 (8 of 16 heads,
i.e. a 512-wide slice of the QKV projections and of the Wp rows).  Each
core computes a partial output-projection for its batch; partials from
the two cores of a batch are summed on the host (cheap 4MB adds), along
with the exactly-factored bias terms:
  - bq is added to Q on-device (affects scores per key-column),
  - bk is dropped (adds a per-query constant to scores: softmax-invariant),
  - bv and bp commute through attention (rows of attn sum to 1):
    y += bv @ Wp + bp, applied on host.

On-chip layout (per core), T=1024, C=1024, DH=64:
  xT   [C, T]   x transposed (host-side transpose)         -> rhs / lhsT
  Q^T  [512, T] = (Wq_s*s)^T x^T  (s=1/8 folded into Wq)   -> scores rhs
  K^T  [512, T]                                            -> scores lhsT
  V    [T, 8, 65] natural layout + ones column             -> ctx lhsT
  S^T  [k-tile 128, q cols <=512]  scores transposed; the softmax
       denominator comes out of the ctx matmul via the ones column of V.
  ctx^T[512, T] normalized context                         -> yproj lhsT

Attention is processed q-chunk-major per head PAIR (heads 2m, 2m+1):
the two heads' scores matmuls have 64-deep contraction at partitions
0-63 / 64-127, so the PE runs them CONCURRENTLY via row-group tiling
(auto tile_position from base_partition).  The causal mask is folded
into the Schraudolph exp as a precomputed bias tensor
(out_i16 = sps*A + (B + A*mask)) via scalar_tensor_tensor on DVE, so
there are no mask matmuls at all; saturation at the negative end gives
0x8000 = -0.0 which masks exactly.  Off-diagonal score tiles use exact
exp on the scalar engine; each PSUM tile is read by exactly one engine
(no same-bank DVE+ACT concurrency).

PSUM budget (8 banks): shared "ps" tag x4 (scores / projections /
yproj halves, all [128,512] f32) + ctx accumulators cps0/cps1 x2 bufs.
"""
import numpy as np

import concourse.bacc as bacc
import concourse.mybir as mybir
import concourse.tile as tile
from concourse.bass_utils import run_bass_kernel_spmd

B, T, C, H, DH = 4, 1024, 1024, 16, 64
P = 128
CS = 512            # per-core head-slice width (8 heads * 64)
F32 = mybir.dt.float32
BF16 = mybir.dt.bfloat16
MM_DTYPE = BF16     # matmul operand dtype
AF = mybir.ActivationFunctionType
ALU = mybir.AluOpType
I16 = mybir.dt.int16
# Schraudolph bf16 exp on DVE: int16(A*x + B) bit-pattern IS bf16 exp(x).
# Saturation at the negative end yields 0x8000 = -0.0, which masks exactly.
SCH_A = 184.66500854  # 128 / ln 2
SCH_B = 16248.0       # 127*128 minus mantissa-interp correction
VPAD = 96           # padded V row stride (elements); 192B-aligned lhsT rows
N_CORES = 8


def build_nc(loop_n=None, mm_dtype=None, phase='full', unroll=None):
    MMD = mm_dtype or MM_DTYPE
    nc = bacc.Bacc("TRN2", target_bir_lowering=False, debug=False,
                   num_devices=N_CORES)
    xT = nc.dram_tensor("xT", [C, T], MMD, kind="ExternalInput")
    wq = nc.dram_tensor("wq", [C, CS], MMD, kind="ExternalInput")
    wk = nc.dram_tensor("wk", [C, CS], MMD, kind="ExternalInput")
    wv = nc.dram_tensor("wv", [C, CS], MMD, kind="ExternalInput")
    wp = nc.dram_tensor("wp", [CS, C], MMD, kind="ExternalInput")
    bq = nc.dram_tensor("bq", [P, 4], F32, kind="ExternalInput")
    mask01 = nc.dram_tensor("mask01", [P, P], MMD, kind="ExternalInput")
    y = nc.dram_tensor("y", [T, C], F32, kind="ExternalOutput")
    dbg = (nc.dram_tensor("dbg", [P, 3, 4224], MMD, kind="ExternalOutput")
           if phase != 'full' else None)

    with tile.TileContext(nc) as tc:
        with (
            tc.tile_pool(name="big", bufs=1) as big,
            tc.tile_pool(name="es_pool", bufs=8) as es_pool,
            tc.tile_pool(name="den_pool", bufs=2) as den_pool,
            tc.tile_pool(name="y_pool", bufs=3) as y_pool,
            tc.tile_pool(name="ps_pool", bufs=4, space="PSUM") as ps_pool,
            tc.tile_pool(name="cps_pool", bufs=2, space="PSUM") as cps_pool,
        ):
            from contextlib import ExitStack
            _ls = ExitStack()
            n_body = 1
            if loop_n:
                n_body = unroll or next(u for u in (8, 4, 2, 1)
                                        if loop_n % u == 0)
                assert loop_n % n_body == 0
                _ls.enter_context(tc.For_i(0, loop_n // n_body, 1))
            for _u in range(n_body):
              xT_sb = big.tile([P, 8, T], MMD)
              wq_sb = big.tile([P, 8, CS], MMD)
              wk_sb = big.tile([P, 8, CS], MMD)
              wv_sb = big.tile([P, 8, CS], MMD)
              wp_sb = big.tile([P, 4, C], MMD)
              bq_sb = big.tile([P, 4], F32)
              m01_sb = big.tile([P, P], MMD)
              qT_sb = big.tile([P, 4, 2, 512], MMD)
              kT_sb = big.tile([P, 4, 2, 512], MMD)
              v_sb = big.tile([P, 8, 8, VPAD], MMD)
              ctxT_sb = big.tile([P, 4, T], MMD)

              # Channel chunks use the "(p c)" split: partition p holds DRAM
              # rows p*8+c, i.e. 8 CONTIGUOUS rows -> one 8-16KB descriptor per
              # partition instead of eight 1-2KB ones.  The contraction is a
              # sum over all 1024 channels, and both matmul operands of every
              # chunk use the SAME permutation, so results are unchanged.
              if phase != 'empty':
                  nc.sync.dma_start(out=bq_sb, in_=bq.ap())
                  nc.sync.dma_start(out=m01_sb, in_=mask01.ap())
                  xT_r = xT.ap().rearrange("(p c) t -> p c t", p=P)
                  wq_r = wq.ap().rearrange("(p c) n -> p c n", p=P)
                  wk_r = wk.ap().rearrange("(p c) n -> p c n", p=P)
                  wv_r = wv.ap().rearrange("(p c) n -> p c n", p=P)
                  # Inputs on the SP queue (wv first: V proj is the first
                  # consumer); wp + y-out on the Act queue, so iteration i's
                  # y DMAs never sit ahead of iteration i+1's inputs in the
                  # same in-order queue.
                  nc.sync.dma_start(out=wv_sb, in_=wv_r)
                  nc.sync.dma_start(out=xT_sb, in_=xT_r)
                  nc.sync.dma_start(out=wq_sb, in_=wq_r)
                  nc.sync.dma_start(out=wk_sb, in_=wk_r)
                  wp_r = wp.ap().rearrange("(k p) n -> p k n", p=P)
                  for kc in range(4):
                      nc.scalar.dma_start(out=wp_sb[:, kc, :], in_=wp_r[:, kc, :])

                  # ---- V natural [T, 512] + ones column per head ----
                  # Engine memset, NOT a scattered DMA: the stride-65
                  # single-element DMA write is not reliably ordered before the
                  # ctx matmul's first read on HW.
                  nc.vector.memset(v_sb[:, :, :, 64], 1.0)
                  for tt in range(8):
                      ps = ps_pool.tile([P, 512], F32, tag="ps", name="vps")
                      for c in range(8):
                          nc.tensor.matmul(
                              ps, xT_sb[:, c, tt * P:(tt + 1) * P], wv_sb[:, c, :],
                              start=(c == 0), stop=(c == 7))
                      dst = v_sb[:, tt, :, 0:64]
                      src = ps.rearrange("p (h d) -> p h d", h=8)
                      if tt % 2 == 0:
                          nc.vector.tensor_copy(dst, src)
                      else:
                          nc.scalar.activation(dst, src, AF.Copy)

              _qk_done = set()

              def qk_proj_group(mc, gi):
                  # gi: 0..3 = (wq,tc0),(wq,tc1),(wk,tc0),(wk,tc1)
                  if (mc, gi) in _qk_done:
                      return
                  _qk_done.add((mc, gi))
                  wsb, outsb, is_q = ((wq_sb, qT_sb, True),
                                      (wk_sb, kT_sb, False))[gi // 2]
                  tc2 = gi % 2
                  ps = ps_pool.tile([P, 512], F32, tag="ps", name="qkps")
                  for c in range(8):
                      nc.tensor.matmul(
                          ps, wsb[:, c, mc * P:(mc + 1) * P],
                          xT_sb[:, c, tc2 * 512:(tc2 + 1) * 512],
                          start=(c == 0), stop=(c == 7))
                  dst = outsb[:, mc, tc2, :]
                  if is_q:
                      nc.vector.tensor_scalar_add(dst, ps, bq_sb[:, mc:mc + 1])
                  else:
                      nc.scalar.activation(dst, ps, AF.Copy)

              def qk_proj(mc):
                  for gi in range(4):
                      qk_proj_group(mc, gi)

              # ---- attention: q-chunk-major, head-pair concurrent ----
              _es_store = {}
              _cps_store = {}

              def S_pair(m, qc, kt):
                  key = (m, qc, kt)
                  if key in _es_store:
                      return
                  diag = kt >= 4 * qc
                  off = (kt - 4 * qc) * P if diag else 0
                  W = 512 - off
                  tc2 = kt // 4
                  c0 = (kt % 4) * P
                  sps_l = []
                  for hr in (0, 1):
                      hp = hr * 64
                      sps = ps_pool.tile([P, W], F32, tag="ps", name="sps")
                      nc.tensor.matmul(sps, kT_sb[hp:hp + 64, m, tc2, c0:c0 + P],
                                       qT_sb[hp:hp + 64, m, qc, off:],
                                       start=True, stop=True)
                      sps_l.append(sps)
                  es_l = []
                  for hr in (0, 1):
                      sps = sps_l[hr]
                      es = es_pool.tile([P, W], MMD, tag="es", name="es")
                      if diag:
                          # exact exp on ScalarE (unmasked), then zero the
                          # upper-triangle of the leading 128 cols with a
                          # cheap bf16 2x-mode DVE multiply (SBUF-only, so
                          # no PSUM bank is touched by two engines).  Exact
                          # exp on near-diagonal keys keeps early queries
                          # (few keys, little averaging) accurate; the
                          # Schraudolph approximation is reserved for far
                          # keys where errors dilute across many keys.
                          nc.scalar.activation(es, sps, AF.Exp)
                          nc.vector.tensor_mul(es[:, 0:P], es[:, 0:P], m01_sb)
                      else:
                          nc.vector.tensor_scalar(
                              es.bitcast(I16), sps,
                              SCH_A, SCH_B, ALU.mult, ALU.add)
                      es_l.append(es)
                  _es_store[key] = es_l

              def C_pair(m, qc, kt):
                  last = 3 if qc == 0 else 7
                  diag = kt >= 4 * qc
                  off = (kt - 4 * qc) * P if diag else 0
                  es_l = _es_store.pop((m, qc, kt))
                  for hr in (0, 1):
                      h = 2 * m + hr
                      if kt == 0:
                          _cps_store[(hr,)] = cps_pool.tile(
                              [65, 512], F32, tag=f"cps{hr}", name="cps")
                      cps = _cps_store[(hr,)]
                      nc.tensor.matmul(cps[:, off:], v_sb[:, kt, h, 0:65],
                                       es_l[hr], start=(kt == 0),
                                       stop=(kt == last))

              def norm_write(m, hr, qc):
                  hp = hr * 64
                  cps = _cps_store.pop((hr,))
                  den = den_pool.tile([1, 512], F32, tag="den", name="den")
                  nc.scalar.activation(den, cps[64:65, :], AF.Copy)
                  recr = den_pool.tile([1, 512], F32, tag="recr", name="recr")
                  # approx_fast needs SBUF input (garbage straight from PSUM);
                  # ~51 ULP, ~5x faster than the 6-cpe exact reciprocal.
                  nc.vector.reciprocal_approx_fast(recr, den)
                  recb = den_pool.tile([64, 512], F32, tag="recb", name="recb")
                  nc.gpsimd.partition_broadcast(recb, recr)
                  nc.vector.tensor_mul(
                      ctxT_sb[hp:hp + 64, m, qc * 512:(qc + 1) * 512],
                      cps[0:64, :], recb)

              def yproj(tt):
                  ps0 = ps_pool.tile([P, 512], F32, tag="ps", name="yps0")
                  ps1 = ps_pool.tile([P, 512], F32, tag="ps", name="yps1")
                  for kc in range(4):
                      lhsT = ctxT_sb[:, kc, tt * P:(tt + 1) * P]
                      nc.tensor.matmul(ps0, lhsT, wp_sb[:, kc, 0:512],
                                       start=(kc == 0), stop=(kc == 3))
                      nc.tensor.matmul(ps1, lhsT, wp_sb[:, kc, 512:],
                                       start=(kc == 0), stop=(kc == 3))
                  ysb = y_pool.tile([P, C], F32, tag="y", name="ysb")
                  nc.scalar.activation(ysb[:, 0:512], ps0, AF.Copy)
                  nc.scalar.activation(ysb[:, 512:], ps1, AF.Copy)
                  nc.scalar.dma_start(
                      out=y.ap()[tt * P:(tt + 1) * P, :], in_=ysb)

              def attention(with_yproj=True):
                  segments = [(m, qc) for m in range(4) for qc in (0, 1)]
                  # fill work to cover exp latency: next pair's projection
                  # groups for m<3; yproj for the last pair.
                  fills = {}
                  for m in range(3):
                      fills[(m, 0, 1)] = lambda m=m: qk_proj_group(m + 1, 0)
                      fills[(m, 0, 2)] = lambda m=m: qk_proj_group(m + 1, 1)
                      fills[(m, 1, 1)] = lambda m=m: qk_proj_group(m + 1, 2)
                      fills[(m, 1, 2)] = lambda m=m: qk_proj_group(m + 1, 3)
                  if with_yproj:
                      for kt in range(1, 5):
                          fills[(3, 1, kt)] = lambda kt=kt: yproj(kt - 1)
                  for idx, (m, qc) in enumerate(segments):
                      last = 3 if qc == 0 else 7
                      for kt in range(last + 1):
                          S_pair(m, qc, kt)
                          if (m, qc, kt) in fills:
                              fills[(m, qc, kt)]()
                          if kt >= 1:
                              C_pair(m, qc, kt - 1)
                      if idx + 1 < len(segments):
                          nm, nqc = segments[idx + 1]
                          S_pair(nm, nqc, 0)
                      C_pair(m, qc, last)
                      norm_write(m, 0, qc)
                      norm_write(m, 1, qc)
                  if with_yproj:
                      for tt in range(4, 8):
                          yproj(tt)

              if phase == 'empty':
                  nc.vector.memset(bq_sb, 0.0)
              elif phase == 'dma':
                  for di, sb_t in enumerate((xT_sb, wq_sb, wk_sb, wv_sb, wp_sb)):
                      nch = sb_t.shape[1]
                      nc.sync.dma_start(
                          out=dbg.ap()[:, 0, di * 64:di * 64 + nch * 8],
                          in_=sb_t[:, :, :8])
              elif phase == 'proj':
                  for mc in range(4):
                      qk_proj(mc)
                  nc.sync.dma_start(out=dbg.ap()[:, 0, :4096],
                                    in_=qT_sb.rearrange("p a b c -> p (a b c)"))
                  nc.sync.dma_start(out=dbg.ap()[:, 1, :4096],
                                    in_=kT_sb.rearrange("p a b c -> p (a b c)"))
                  nc.sync.dma_start(out=dbg.ap()[:, 2, :4160],
                                    in_=v_sb[:, :, :, 0:65]
                                    .rearrange("p a b c -> p (a b c)"))
              elif phase == 'attn':
                  qk_proj(0)
                  attention(with_yproj=False)
                  nc.sync.dma_start(out=dbg.ap()[:, 0, :4096],
                                    in_=ctxT_sb.rearrange("p a b -> p (a b)"))
              else:
                  qk_proj(0)
                  attention(with_yproj=True)
            _ls.close()
    nc.compile()
    return nc


_NC = None


def _get_nc():
    global _NC
    if _NC is None:
        _NC = build_nc()
    return _NC


def make_in_maps(x, Wq, bq, Wk, Wv, Wp, mm_dtype=None):
    """Per-core input dicts."""
    import ml_dtypes
    MMD = mm_dtype or MM_DTYPE
    cvt = ((lambda a: np.ascontiguousarray(a).astype(ml_dtypes.bfloat16))
           if MMD == BF16 else np.ascontiguousarray)
    valid = (np.arange(P)[None, :] >= np.arange(P)[:, None])
    mask01 = cvt(valid.astype(np.float32))
    in_maps = []
    for core in range(N_CORES):
        b = core // 2
        g = core % 2
        cs = slice(g * CS, (g + 1) * CS)
        in_maps.append(dict(
            xT=cvt(x[b].T),
            wq=cvt(Wq[:, cs] * np.float32(0.125)),
            wk=cvt(Wk[:, cs]),
            wv=cvt(Wv[:, cs]),
            wp=cvt(Wp[cs, :]),
            bq=np.ascontiguousarray((bq[cs] * np.float32(0.125))
                                    .reshape(4, P).T),
            mask01=mask01,
        ))
    return in_maps


def combine(parts, Wq, bv, Wp, bp):
    """parts: list of 8 per-core partial y arrays -> full [B, T, C] output."""
    out = np.stack([parts[2 * b] + parts[2 * b + 1] for b in range(B)])
    out += (bv @ Wp + bp)[None, None, :]
    return out.astype(np.float32)


def kernel(**inputs):
    x = np.asarray(inputs["x"], np.float32)
    Wq = np.asarray(inputs["Wq"], np.float32)
    bq = np.asarray(inputs["bq"], np.float32)
    Wk = np.asarray(inputs["Wk"], np.float32)
    Wv = np.asarray(inputs["Wv"], np.float32)
    Wp = np.asarray(inputs["Wp"], np.float32)
    bv = np.asarray(inputs["bv"], np.float32)
    bp = np.asarray(inputs["bp"], np.float32)
    # bk intentionally unused: it shifts every score of a query row by the
    # same amount, which softmax cancels exactly.

    nc = _get_nc()
    in_maps = make_in_maps(x, Wq, bq, Wk, Wv, Wp)
    res = run_bass_kernel_spmd(nc, in_maps, core_ids=list(range(N_CORES)))
    parts = [res.results[c]["y"] for c in range(N_CORES)]
    return combine(parts, Wq, bv, Wp, bp)


# revision 41
# speedup vs baseline: 1.0808x; 1.0808x over previous
"""Multi-head causal attention (GPT-2 style) on 8 TRN2 NeuronCores.

Sharding: core i handles batch i//2 and head-group i%2 (8 of 16 heads,
i.e. a 512-wide slice of the QKV projections and of the Wp rows).  Each
core computes a partial output-projection for its batch; partials from
the two cores of a batch are summed on the host (cheap 4MB adds), along
with the exactly-factored bias terms:
  - bq is added to Q on-device (affects scores per key-column),
  - bk is dropped (adds a per-query constant to scores: softmax-invariant),
  - bv and bp commute through attention (rows of attn sum to 1):
    y += bv @ Wp + bp, applied on host.

On-chip layout (per core), T=1024, C=1024, DH=64:
  xT   [C, T]   x transposed (host-side transpose)         -> rhs / lhsT
  Q^T  [512, T] = (Wq_s*s)^T x^T  (s=1/8 folded into Wq)   -> scores rhs
  K^T  [512, T]                                            -> scores lhsT
  V    [T, 8, 65] natural layout + ones column             -> ctx lhsT
  S^T  [k-tile 128, q cols <=512]  scores transposed; the softmax
       denominator comes out of the ctx matmul via the ones column of V.
  ctx^T[512, T] normalized context                         -> yproj lhsT

Attention is processed q-chunk-major per head PAIR (heads 2m, 2m+1):
the two heads' scores matmuls have 64-deep contraction at partitions
0-63 / 64-127, so the PE runs them CONCURRENTLY via row-group tiling
(auto tile_position from base_partition; measured 341ns vs 1036ns
serial for an N=512 pair).  There are NO mask matmuls: diagonal score
tiles get exact exp on ScalarE followed by one bf16 2x-mode DVE
multiply with a 0/1 mask (SBUF-only, so no PSUM bank sees two
engines); off-diagonal tiles alternate whole-pair Schraudolph-on-DVE /
exact-exp-on-ScalarE by kt parity.  ctx matmuls run TWO steps behind
scores so the in-order PE stream never waits on a just-computed exp.
Next-pair QK projection groups and the output projection are woven
into attention steps as PE filler.

All bf16 operands ship in ONE ~6.2MB blob DMA per iteration (split
DMAs serialize on the two HWDGE rings: 11 DMAs measured 59us/iter vs
20us for the blob), double-buffered so it prefetches a full iteration
ahead; y rides the Act ring in bf16.

PSUM budget (8 banks): shared "ps" tag x2 bufs (score pairs /
projection pairs / yproj halves, all [128,2,512] f32 = 2 banks each,
padded so both halves are bank-aligned) + ctx accumulators
cps0/cps1 x2 bufs.
"""
import numpy as np

import concourse.bacc as bacc
import concourse.mybir as mybir
import concourse.tile as tile
from concourse.bass_utils import run_bass_kernel_spmd

B, T, C, H, DH = 4, 1024, 1024, 16, 64
P = 128
CS = 512            # per-core head-slice width (8 heads * 64)
F32 = mybir.dt.float32
BF16 = mybir.dt.bfloat16
MM_DTYPE = BF16     # matmul operand dtype
AF = mybir.ActivationFunctionType
ALU = mybir.AluOpType
I16 = mybir.dt.int16
# Schraudolph bf16 exp on DVE: int16(A*x + B) bit-pattern IS bf16 exp(x).
# Saturation at the negative end yields 0x8000 = -0.0, which masks exactly.
SCH_A = 184.66500854  # 128 / ln 2
SCH_B = 16248.0       # 127*128 minus mantissa-interp correction
VPAD = 96           # padded V row stride (elements); 192B-aligned lhsT rows
N_CORES = 8


# Packed input blob layout (per partition, bf16 elements): all matmul
# operands ride in ONE HBM->SBUF DMA per iteration (11 small DMAs serialize
# to ~59us on the HWDGE rings; one 6.2MB DMA streams at ~270GB/s and double-
# buffers cleanly under compute).
BLOB_XT = 0                      # [8, 1024]  x^T  "(p c) t"
BLOB_WQ = BLOB_XT + 8 * 1024     # [8, 512]
BLOB_WK = BLOB_WQ + 8 * 512
BLOB_WV = BLOB_WK + 8 * 512
BLOB_WP = BLOB_WV + 8 * 512      # [4, 1024]  "(k p) n"
BLOB_M01 = BLOB_WP + 4 * 1024    # [2, 128] (same mask twice: one merged
BLOB_LEN = BLOB_M01 + 2 * P      #  [P,2,128] multiply masks both heads)


def build_nc(loop_n=None, mm_dtype=None, phase='full', unroll=None):
    MMD = mm_dtype or MM_DTYPE
    nc = bacc.Bacc("TRN2", target_bir_lowering=False, debug=False,
                   num_devices=N_CORES)
    blob = nc.dram_tensor("blob", [P, BLOB_LEN], MMD, kind="ExternalInput")
    bq = nc.dram_tensor("bq", [P, 4], F32, kind="ExternalInput")
    y = nc.dram_tensor("y", [T, C], MMD, kind="ExternalOutput")
    dbg = (nc.dram_tensor("dbg", [P, 3, 4224], MMD, kind="ExternalOutput")
           if phase not in ('full', 'noy') else None)

    with tile.TileContext(nc) as tc:
        with (
            tc.tile_pool(name="big", bufs=1) as big,
            tc.tile_pool(name="es_pool", bufs=8) as es_pool,
            tc.tile_pool(name="den_pool", bufs=2) as den_pool,
            tc.tile_pool(name="y_pool", bufs=3) as y_pool,
            tc.tile_pool(name="ps_pool", bufs=2, space="PSUM") as ps_pool,
            tc.tile_pool(name="cps_pool", bufs=2, space="PSUM") as cps_pool,
        ):
            from contextlib import ExitStack
            _ls = ExitStack()
            n_body = 1
            if loop_n:
                n_body = unroll or next(u for u in (8, 4, 2, 1)
                                        if loop_n % u == 0)
                assert loop_n % n_body == 0
                _ls.enter_context(tc.For_i(0, loop_n // n_body, 1))
            for _u in range(n_body):
              blob_sb = big.tile([P, BLOB_LEN], MMD, bufs=2)
              bq_sb = big.tile([P, 4], F32, bufs=2)
              qT_sb = big.tile([P, 4, 2, 512], MMD)
              kT_sb = big.tile([P, 4, 2, 512], MMD)
              v_sb = big.tile([P, 8, 8, VPAD], MMD)
              ctxT_sb = big.tile([P, 4, T], MMD)

              xT_sb = blob_sb[:, BLOB_XT:BLOB_WQ].rearrange(
                  "p (c t) -> p c t", t=T)
              wq_sb = blob_sb[:, BLOB_WQ:BLOB_WK].rearrange(
                  "p (c n) -> p c n", n=CS)
              wk_sb = blob_sb[:, BLOB_WK:BLOB_WV].rearrange(
                  "p (c n) -> p c n", n=CS)
              wv_sb = blob_sb[:, BLOB_WV:BLOB_WP].rearrange(
                  "p (c n) -> p c n", n=CS)
              wp_sb = blob_sb[:, BLOB_WP:BLOB_M01].rearrange(
                  "p (k n) -> p k n", n=C)
              m01_sb = blob_sb[:, BLOB_M01:BLOB_LEN].rearrange(
                  "p (a b) -> p a b", a=2)

              # Channel chunks use the "(p c)" split: partition p holds DRAM
              # rows p*8+c, i.e. 8 CONTIGUOUS rows per operand chunk; the
              # contraction sums over all 1024 channels and both matmul
              # operands of every chunk use the SAME permutation, so results
              # are unchanged.  All operands ship in ONE blob DMA (SP queue);
              # y-out rides the Act queue so output DMAs never sit ahead of
              # the next iteration's inputs in the same in-order queue.
              if phase not in ('empty', 'nodma'):
                  nc.sync.dma_start(out=blob_sb, in_=blob.ap())
                  nc.sync.dma_start(out=bq_sb, in_=bq.ap())

                  # ---- V natural [T, 512] + ones column per head ----
                  # Engine memset, NOT a scattered DMA: the stride-65
                  # single-element DMA write is not reliably ordered before the
                  # ctx matmul's first read on HW.
                  nc.vector.memset(v_sb[:, :, :, 64], 1.0)
                  for tj in range(4):
                      ps = ps_pool.tile([P, 2, 512], F32, tag="ps", name="vps")
                      for half in range(2):
                          tt = 2 * tj + half
                          for c in range(8):
                              nc.tensor.matmul(
                                  ps[:, half, :],
                                  xT_sb[:, c, tt * P:(tt + 1) * P],
                                  wv_sb[:, c, :],
                                  start=(c == 0), stop=(c == 7))
                      dst = v_sb[:, 2 * tj:2 * tj + 2, :, 0:64]
                      src = ps.rearrange("p a (h d) -> p a h d", h=8)
                      if tj % 2 == 0:
                          nc.vector.tensor_copy(dst, src)
                      else:
                          nc.scalar.activation(dst, src, AF.Copy)

              _qk_done = set()

              def qk_proj_group(mc, g2):
                  # g2: 0 = wq (both t-chunks), 1 = wk (both t-chunks)
                  if (mc, g2) in _qk_done:
                      return
                  _qk_done.add((mc, g2))
                  wsb, outsb, is_q = ((wq_sb, qT_sb, True),
                                      (wk_sb, kT_sb, False))[g2]
                  ps = ps_pool.tile([P, 2, 512], F32, tag="ps", name="qkps")
                  for tc2 in range(2):
                      for c in range(8):
                          nc.tensor.matmul(
                              ps[:, tc2, :], wsb[:, c, mc * P:(mc + 1) * P],
                              xT_sb[:, c, tc2 * 512:(tc2 + 1) * 512],
                              start=(c == 0), stop=(c == 7))
                  dst = outsb[:, mc, :, :]
                  if is_q:
                      nc.vector.tensor_scalar_add(dst, ps, bq_sb[:, mc:mc + 1])
                  else:
                      nc.scalar.activation(dst, ps, AF.Copy)

              def qk_proj(mc):
                  for g2 in range(2):
                      qk_proj_group(mc, g2)

              # ---- attention: q-chunk-major, head-pair concurrent ----
              _es_store = {}
              _cps_store = {}

              def S_pair(m, qc, kt):
                  key = (m, qc, kt)
                  if key in _es_store:
                      return
                  diag = kt >= 4 * qc
                  off = (kt - 4 * qc) * P if diag else 0
                  W = 512 - off
                  tc2 = kt // 4
                  c0 = (kt % 4) * P
                  # Both heads of the pair share one 2-bank PSUM tile; the
                  # two scores matmuls have 64-deep contraction at partitions
                  # 0-63 / 64-127, so the PE runs them concurrently via
                  # row-group tiling (tile_position auto-derived).
                  sps = ps_pool.tile([P, 2, W], F32, tag="ps", name="sps",
                                     padded_shape=(P, 2, 512))
                  for hr in (0, 1):
                      hp = hr * 64
                      nc.tensor.matmul(sps[:, hr, :],
                                       kT_sb[hp:hp + 64, m, tc2, c0:c0 + P],
                                       qT_sb[hp:hp + 64, m, qc, off:],
                                       start=True, stop=True)
                  es = es_pool.tile([P, 2, W], MMD, tag="es", name="es")
                  if phase in ('noexp', 'noboth'):
                      nc.gpsimd.memset(es, 0.002)
                  elif diag:
                      # exact exp on ScalarE (unmasked), then zero the
                      # upper-triangle of both heads' leading 128 cols with
                      # ONE bf16 2x-mode DVE multiply (SBUF-only, so no PSUM
                      # bank is touched by two engines).  Exact exp on
                      # near-diagonal keys keeps early queries (few keys,
                      # little averaging) accurate; Schraudolph is reserved
                      # for far keys where errors dilute across many keys.
                      nc.scalar.activation(es, sps, AF.Exp)
                      nc.vector.tensor_mul(es[:, :, 0:P], es[:, :, 0:P],
                                           m01_sb)
                  elif kt % 2 == 0:
                      nc.scalar.activation(es, sps, AF.Exp)
                  else:
                      nc.vector.tensor_scalar(
                          es.bitcast(I16), sps,
                          SCH_A, SCH_B, ALU.mult, ALU.add)
                  _es_store[key] = es

              def C_pair(m, qc, kt):
                  last = 3 if qc == 0 else 7
                  diag = kt >= 4 * qc
                  off = (kt - 4 * qc) * P if diag else 0
                  es = _es_store.pop((m, qc, kt))
                  for hr in (0, 1):
                      h = 2 * m + hr
                      if kt == 0:
                          _cps_store[(hr,)] = cps_pool.tile(
                              [65, 512], F32, tag=f"cps{hr}", name="cps")
                      cps = _cps_store[(hr,)]
                      nc.tensor.matmul(cps[:, off:], v_sb[:, kt, h, 0:65],
                                       es[:, hr, :], start=(kt == 0),
                                       stop=(kt == last))

              def norm_write(m, hr, qc):
                  hp = hr * 64
                  cps = _cps_store.pop((hr,))
                  if phase in ('nonorm', 'noboth'):
                      return
                  den = den_pool.tile([1, 512], F32, tag="den", name="den")
                  nc.scalar.activation(den, cps[64:65, :], AF.Copy)
                  recr = den_pool.tile([1, 512], F32, tag="recr", name="recr")
                  # approx_fast needs SBUF input (garbage straight from PSUM);
                  # ~51 ULP, ~5x faster than the 6-cpe exact reciprocal.
                  nc.vector.reciprocal_approx_fast(recr, den)
                  recb = den_pool.tile([64, 512], F32, tag="recb", name="recb")
                  nc.gpsimd.partition_broadcast(recb, recr)
                  nc.vector.tensor_mul(
                      ctxT_sb[hp:hp + 64, m, qc * 512:(qc + 1) * 512],
                      cps[0:64, :], recb)

              def yproj(tt):
                  ps = ps_pool.tile([P, 2, 512], F32, tag="ps", name="yps")
                  for kc in range(4):
                      lhsT = ctxT_sb[:, kc, tt * P:(tt + 1) * P]
                      nc.tensor.matmul(ps[:, 0, :], lhsT, wp_sb[:, kc, 0:512],
                                       start=(kc == 0), stop=(kc == 3))
                      nc.tensor.matmul(ps[:, 1, :], lhsT, wp_sb[:, kc, 512:],
                                       start=(kc == 0), stop=(kc == 3))
                  ysb = y_pool.tile([P, C], MMD, tag="y", name="ysb")
                  if tt % 2 == 0:
                      nc.scalar.activation(
                          ysb.rearrange("p (a b) -> p a b", a=2), ps, AF.Copy)
                  else:
                      nc.vector.tensor_copy(
                          ysb.rearrange("p (a b) -> p a b", a=2), ps)
                  if phase != 'noy':
                      nc.scalar.dma_start(
                          out=y.ap()[tt * P:(tt + 1) * P, :], in_=ysb)

              def attention(with_yproj=True):
                  segments = [(m, qc) for m in range(4) for qc in (0, 1)]
                  # fill work to cover exp latency: next pair's projection
                  # groups for m<3; yproj for the last pair.
                  fills = {}
                  for m in range(3):
                      fills[(m, 0, 2)] = lambda m=m: qk_proj_group(m + 1, 0)
                      fills[(m, 1, 2)] = lambda m=m: qk_proj_group(m + 1, 1)
                  if with_yproj:
                      for kt in range(2, 6):
                          fills[(3, 1, kt)] = lambda kt=kt: yproj(kt - 2)
                  # ctx runs TWO steps behind scores: exp(kt) gets a full
                  # extra step of engine time before ctx(kt) needs it, so the
                  # in-order PE stream never waits on a just-computed exp.
                  for idx, (m, qc) in enumerate(segments):
                      last = 3 if qc == 0 else 7
                      S_pair(m, qc, 0)
                      S_pair(m, qc, 1)
                      for kt in range(2, last + 1):
                          S_pair(m, qc, kt)
                          if (m, qc, kt) in fills:
                              fills[(m, qc, kt)]()
                          C_pair(m, qc, kt - 2)
                      if idx + 1 < len(segments):
                          nm, nqc = segments[idx + 1]
                          S_pair(nm, nqc, 0)
                      C_pair(m, qc, last - 1)
                      if idx + 1 < len(segments):
                          S_pair(nm, nqc, 1)
                      C_pair(m, qc, last)
                      norm_write(m, 0, qc)
                      norm_write(m, 1, qc)
                  if with_yproj:
                      for tt in range(4, 8):
                          yproj(tt)

              if phase == 'empty':
                  nc.vector.memset(bq_sb, 0.0)
              elif phase == 'dma':
                  for di, sb_t in enumerate((xT_sb, wq_sb, wk_sb, wv_sb, wp_sb)):
                      nch = sb_t.shape[1]
                      nc.sync.dma_start(
                          out=dbg.ap()[:, 0, di * 64:di * 64 + nch * 8],
                          in_=sb_t[:, :, :8])
              elif phase == 'proj':
                  for mc in range(4):
                      qk_proj(mc)
                  nc.sync.dma_start(out=dbg.ap()[:, 0, :4096],
                                    in_=qT_sb.rearrange("p a b c -> p (a b c)"))
                  nc.sync.dma_start(out=dbg.ap()[:, 1, :4096],
                                    in_=kT_sb.rearrange("p a b c -> p (a b c)"))
                  for kt8 in range(8):
                      nc.sync.dma_start(
                          out=dbg.ap()[:, 2, kt8 * 520:kt8 * 520 + 520]
                          .rearrange("p (b c) -> p b c", b=8),
                          in_=v_sb[:, kt8, :, 0:65])
              elif phase == 'attn':
                  qk_proj(0)
                  attention(with_yproj=False)
                  nc.sync.dma_start(out=dbg.ap()[:, 0, :4096],
                                    in_=ctxT_sb.rearrange("p a b -> p (a b)"))
              elif phase in ('a_nodbg', 'noexp', 'nonorm', 'noboth'):
                  qk_proj(0)
                  attention(with_yproj=False)
                  nc.sync.dma_start(out=dbg.ap()[:, 0, :1],
                                    in_=ctxT_sb[:, 0, 0:1])
              else:
                  qk_proj(0)
                  attention(with_yproj=True)
            _ls.close()
    nc.compile()
    return nc


_NC = None


def _get_nc():
    global _NC
    if _NC is None:
        _NC = build_nc()
    return _NC


def make_in_maps(x, Wq, bq, Wk, Wv, Wp, mm_dtype=None):
    """Per-core input dicts."""
    import ml_dtypes
    MMD = mm_dtype or MM_DTYPE
    cvt = ((lambda a: np.ascontiguousarray(a).astype(ml_dtypes.bfloat16))
           if MMD == BF16 else np.ascontiguousarray)
    valid = (np.arange(P)[None, :] >= np.arange(P)[:, None])
    mask01 = valid.astype(np.float32)
    in_maps = []
    for core in range(N_CORES):
        b = core // 2
        g = core % 2
        cs = slice(g * CS, (g + 1) * CS)
        blob = np.concatenate([
            x[b].T.reshape(P, 8 * 1024),
            (Wq[:, cs] * np.float32(0.125)).reshape(P, 8 * 512),
            Wk[:, cs].reshape(P, 8 * 512),
            Wv[:, cs].reshape(P, 8 * 512),
            Wp[cs, :].reshape(4, P, 1024).transpose(1, 0, 2).reshape(P, 4096),
            mask01, mask01,
        ], axis=1)
        assert blob.shape == (P, BLOB_LEN)
        in_maps.append(dict(
            blob=cvt(blob),
            bq=np.ascontiguousarray((bq[cs] * np.float32(0.125))
                                    .reshape(4, P).T),
        ))
    return in_maps


def combine(parts, Wq, bv, Wp, bp):
    """parts: list of 8 per-core partial y arrays -> full [B, T, C] output."""
    out = np.stack([parts[2 * b].astype(np.float32)
                    + parts[2 * b + 1].astype(np.float32)
                    for b in range(B)])
    out += (bv @ Wp + bp)[None, None, :]
    return out.astype(np.float32)


def kernel(**inputs):
    x = np.asarray(inputs["x"], np.float32)
    Wq = np.asarray(inputs["Wq"], np.float32)
    bq = np.asarray(inputs["bq"], np.float32)
    Wk = np.asarray(inputs["Wk"], np.float32)
    Wv = np.asarray(inputs["Wv"], np.float32)
    Wp = np.asarray(inputs["Wp"], np.float32)
    bv = np.asarray(inputs["bv"], np.float32)
    bp = np.asarray(inputs["bp"], np.float32)
    # bk intentionally unused: it shifts every score of a query row by the
    # same amount, which softmax cancels exactly.

    nc = _get_nc()
    in_maps = make_in_maps(x, Wq, bq, Wk, Wv, Wp)
    res = run_bass_kernel_spmd(nc, in_maps, core_ids=list(range(N_CORES)))
    parts = [res.results[c]["y"] for c in range(N_CORES)]
    return combine(parts, Wq, bv, Wp, bp)


# revision 43
# speedup vs baseline: 1.0831x; 1.0022x over previous
"""Multi-head causal attention (GPT-2 style) on 8 TRN2 NeuronCores.

Sharding: core i handles batch i//2 and head-group i%2 (8 of 16 heads,
i.e. a 512-wide slice of the QKV projections and of the Wp rows).  Each
core computes a partial output-projection for its batch; partials from
the two cores of a batch are summed on the host (cheap 4MB adds), along
with the exactly-factored bias terms:
  - bq is added to Q on-device (affects scores per key-column),
  - bk is dropped (adds a per-query constant to scores: softmax-invariant),
  - bv and bp commute through attention (rows of attn sum to 1):
    y += bv @ Wp + bp, applied on host.

On-chip layout (per core), T=1024, C=1024, DH=64:
  xT   [C, T]   x transposed (host-side transpose)         -> rhs / lhsT
  Q^T  [512, T] = (Wq_s*s)^T x^T  (s=1/8 folded into Wq)   -> scores rhs
  K^T  [512, T]                                            -> scores lhsT
  V    [T, 8, 65] natural layout + ones column             -> ctx lhsT
  S^T  [k-tile 128, q cols <=512]  scores transposed; the softmax
       denominator comes out of the ctx matmul via the ones column of V.
  ctx^T[512, T] normalized context                         -> yproj lhsT

Attention is processed q-chunk-major per head PAIR (heads 2m, 2m+1):
the two heads' scores matmuls have 64-deep contraction at partitions
0-63 / 64-127, so the PE runs them CONCURRENTLY via row-group tiling
(auto tile_position from base_partition; measured 341ns vs 1036ns
serial for an N=512 pair).  There are NO mask matmuls: diagonal score
tiles get exact exp on ScalarE followed by one bf16 2x-mode DVE
multiply with a 0/1 mask (SBUF-only, so no PSUM bank sees two
engines); off-diagonal tiles alternate whole-pair Schraudolph-on-DVE /
exact-exp-on-ScalarE by kt parity.  ctx matmuls run TWO steps behind
scores so the in-order PE stream never waits on a just-computed exp.
Next-pair QK projection groups and the output projection are woven
into attention steps as PE filler.

All bf16 operands ship in ONE ~6.2MB blob DMA per iteration (split
DMAs serialize on the two HWDGE rings: 11 DMAs measured 59us/iter vs
20us for the blob), double-buffered so it prefetches a full iteration
ahead; y rides the Act ring in bf16.

PSUM budget (8 banks): shared "ps" tag x2 bufs (score pairs /
projection pairs / yproj halves, all [128,2,512] f32 = 2 banks each,
padded so both halves are bank-aligned) + ctx accumulators
cps0/cps1 x2 bufs.
"""
import numpy as np

import concourse.bacc as bacc
import concourse.mybir as mybir
import concourse.tile as tile
from concourse.bass_utils import run_bass_kernel_spmd

B, T, C, H, DH = 4, 1024, 1024, 16, 64
P = 128
CS = 512            # per-core head-slice width (8 heads * 64)
F32 = mybir.dt.float32
BF16 = mybir.dt.bfloat16
MM_DTYPE = BF16     # matmul operand dtype
AF = mybir.ActivationFunctionType
ALU = mybir.AluOpType
I16 = mybir.dt.int16
# Schraudolph bf16 exp on DVE: int16(A*x + B) bit-pattern IS bf16 exp(x).
# Saturation at the negative end yields 0x8000 = -0.0, which masks exactly.
SCH_A = 184.66500854  # 128 / ln 2
SCH_B = 16248.0       # 127*128 minus mantissa-interp correction
VPAD = 96           # padded V row stride (elements); 192B-aligned lhsT rows
N_CORES = 8


# Packed input blob layout (per partition, bf16 elements): all matmul
# operands ride in ONE HBM->SBUF DMA per iteration (11 small DMAs serialize
# to ~59us on the HWDGE rings; one 6.2MB DMA streams at ~270GB/s and double-
# buffers cleanly under compute).
BLOB_XT = 0                      # [8, 1024]  x^T  "(p c) t"
BLOB_WQ = BLOB_XT + 8 * 1024     # [8, 512]
BLOB_WK = BLOB_WQ + 8 * 512
BLOB_WV = BLOB_WK + 8 * 512
BLOB_WP = BLOB_WV + 8 * 512      # [4, 1024]  "(k p) n"
BLOB_M01 = BLOB_WP + 4 * 1024    # [2, 128] (same mask twice: one merged
BLOB_LEN = BLOB_M01 + 2 * P      #  [P,2,128] multiply masks both heads)


def build_nc(loop_n=None, mm_dtype=None, phase='full', unroll=None):
    MMD = mm_dtype or MM_DTYPE
    nc = bacc.Bacc("TRN2", target_bir_lowering=False, debug=False,
                   num_devices=N_CORES)
    blob = nc.dram_tensor("blob", [P, BLOB_LEN], MMD, kind="ExternalInput")
    bq = nc.dram_tensor("bq", [P, 4], F32, kind="ExternalInput")
    y = nc.dram_tensor("y", [T, C], MMD, kind="ExternalOutput")
    dbg = (nc.dram_tensor("dbg", [P, 3, 4224], MMD, kind="ExternalOutput")
           if phase not in ('full', 'noy') else None)

    with tile.TileContext(nc) as tc:
        with (
            tc.tile_pool(name="big", bufs=1) as big,
            tc.tile_pool(name="es_pool", bufs=8) as es_pool,
            tc.tile_pool(name="den_pool", bufs=2) as den_pool,
            tc.tile_pool(name="y_pool", bufs=3) as y_pool,
            tc.tile_pool(name="ps_pool", bufs=2, space="PSUM") as ps_pool,
            tc.tile_pool(name="cps_pool", bufs=2, space="PSUM") as cps_pool,
        ):
            from contextlib import ExitStack
            _ls = ExitStack()
            n_body = 1
            if loop_n:
                n_body = unroll or next(u for u in (8, 4, 2, 1)
                                        if loop_n % u == 0)
                assert loop_n % n_body == 0
                _ls.enter_context(tc.For_i(0, loop_n // n_body, 1))
            for _u in range(n_body):
              blob_sb = big.tile([P, BLOB_LEN], MMD, bufs=2)
              bq_sb = big.tile([P, 4], F32, bufs=2)
              qT_sb = big.tile([P, 4, 2, 512], MMD)
              kT_sb = big.tile([P, 4, 2, 512], MMD)
              v_sb = big.tile([P, 8, 8, VPAD], MMD)
              ctxT_sb = big.tile([P, 4, T], MMD)

              xT_sb = blob_sb[:, BLOB_XT:BLOB_WQ].rearrange(
                  "p (c t) -> p c t", t=T)
              wq_sb = blob_sb[:, BLOB_WQ:BLOB_WK].rearrange(
                  "p (c n) -> p c n", n=CS)
              wk_sb = blob_sb[:, BLOB_WK:BLOB_WV].rearrange(
                  "p (c n) -> p c n", n=CS)
              wv_sb = blob_sb[:, BLOB_WV:BLOB_WP].rearrange(
                  "p (c n) -> p c n", n=CS)
              wp_sb = blob_sb[:, BLOB_WP:BLOB_M01].rearrange(
                  "p (k n) -> p k n", n=C)
              m01_sb = blob_sb[:, BLOB_M01:BLOB_LEN].rearrange(
                  "p (a b) -> p a b", a=2)

              # Channel chunks use the "(p c)" split: partition p holds DRAM
              # rows p*8+c, i.e. 8 CONTIGUOUS rows per operand chunk; the
              # contraction sums over all 1024 channels and both matmul
              # operands of every chunk use the SAME permutation, so results
              # are unchanged.  All operands ship in ONE blob DMA (SP queue);
              # y-out rides the Act queue so output DMAs never sit ahead of
              # the next iteration's inputs in the same in-order queue.
              if phase not in ('empty', 'nodma'):
                  nc.sync.dma_start(out=blob_sb, in_=blob.ap())
                  nc.sync.dma_start(out=bq_sb, in_=bq.ap())

                  # ---- V natural [T, 512] + ones column per head ----
                  # Engine memset, NOT a scattered DMA: the stride-65
                  # single-element DMA write is not reliably ordered before the
                  # ctx matmul's first read on HW.
                  nc.vector.memset(v_sb[:, :, :, 64], 1.0)
                  for tj in range(4):
                      ps = ps_pool.tile([P, 2, 512], F32, tag="ps", name="vps")
                      for half in range(2):
                          tt = 2 * tj + half
                          for c in range(8):
                              nc.tensor.matmul(
                                  ps[:, half, :],
                                  xT_sb[:, c, tt * P:(tt + 1) * P],
                                  wv_sb[:, c, :],
                                  start=(c == 0), stop=(c == 7))
                      dst = v_sb[:, 2 * tj:2 * tj + 2, :, 0:64]
                      src = ps.rearrange("p a (h d) -> p a h d", h=8)
                      if tj % 2 == 0:
                          nc.vector.tensor_copy(dst, src)
                      else:
                          nc.scalar.activation(dst, src, AF.Copy)

              _qk_done = set()

              def qk_proj_group(mc, g2):
                  # g2: 0 = wq (both t-chunks), 1 = wk (both t-chunks)
                  if (mc, g2) in _qk_done:
                      return
                  _qk_done.add((mc, g2))
                  wsb, outsb, is_q = ((wq_sb, qT_sb, True),
                                      (wk_sb, kT_sb, False))[g2]
                  ps = ps_pool.tile([P, 2, 512], F32, tag="ps", name="qkps")
                  for tc2 in range(2):
                      for c in range(8):
                          nc.tensor.matmul(
                              ps[:, tc2, :], wsb[:, c, mc * P:(mc + 1) * P],
                              xT_sb[:, c, tc2 * 512:(tc2 + 1) * 512],
                              start=(c == 0), stop=(c == 7))
                  dst = outsb[:, mc, :, :]
                  if is_q:
                      nc.vector.tensor_scalar_add(dst, ps, bq_sb[:, mc:mc + 1])
                  else:
                      nc.scalar.activation(dst, ps, AF.Copy)

              def qk_proj(mc):
                  for g2 in range(2):
                      qk_proj_group(mc, g2)

              # ---- attention: q-chunk-major, head-pair concurrent ----
              _es_store = {}
              _cps_store = {}

              def S_pair(m, qc, kt):
                  key = (m, qc, kt)
                  if key in _es_store:
                      return
                  diag = kt >= 4 * qc
                  off = (kt - 4 * qc) * P if diag else 0
                  W = 512 - off
                  tc2 = kt // 4
                  c0 = (kt % 4) * P
                  # Both heads of the pair share one 2-bank PSUM tile; the
                  # two scores matmuls have 64-deep contraction at partitions
                  # 0-63 / 64-127, so the PE runs them concurrently via
                  # row-group tiling (tile_position auto-derived).
                  sps = ps_pool.tile([P, 2, W], F32, tag="ps", name="sps",
                                     padded_shape=(P, 2, 512))
                  for hr in (0, 1):
                      hp = hr * 64
                      nc.tensor.matmul(sps[:, hr, :],
                                       kT_sb[hp:hp + 64, m, tc2, c0:c0 + P],
                                       qT_sb[hp:hp + 64, m, qc, off:],
                                       start=True, stop=True)
                  es = es_pool.tile([P, 2, W], MMD, tag="es", name="es")
                  if phase in ('noexp', 'noboth'):
                      nc.gpsimd.memset(es, 0.002)
                  elif diag:
                      # exact exp on ScalarE (unmasked), then zero the
                      # upper-triangle of both heads' leading 128 cols with
                      # ONE bf16 2x-mode DVE multiply (SBUF-only, so no PSUM
                      # bank is touched by two engines).  Exact exp on
                      # near-diagonal keys keeps early queries (few keys,
                      # little averaging) accurate; Schraudolph is reserved
                      # for far keys where errors dilute across many keys.
                      nc.scalar.activation(es, sps, AF.Exp)
                      nc.vector.tensor_mul(es[:, :, 0:P], es[:, :, 0:P],
                                           m01_sb)
                  elif kt % 2 == 0:
                      nc.scalar.activation(es, sps, AF.Exp)
                  else:
                      nc.vector.tensor_scalar(
                          es.bitcast(I16), sps,
                          SCH_A, SCH_B, ALU.mult, ALU.add)
                  _es_store[key] = es

              def C_pair(m, qc, kt):
                  last = 3 if qc == 0 else 7
                  diag = kt >= 4 * qc
                  off = (kt - 4 * qc) * P if diag else 0
                  es = _es_store.pop((m, qc, kt))
                  for hr in (0, 1):
                      h = 2 * m + hr
                      if kt == 0:
                          _cps_store[(hr,)] = cps_pool.tile(
                              [65, 512], F32, tag=f"cps{hr}", name="cps")
                      cps = _cps_store[(hr,)]
                      nc.tensor.matmul(cps[:, off:], v_sb[:, kt, h, 0:65],
                                       es[:, hr, :], start=(kt == 0),
                                       stop=(kt == last))

              def norm_write(m, hr, qc):
                  hp = hr * 64
                  cps = _cps_store.pop((hr,))
                  if phase in ('nonorm', 'noboth'):
                      return
                  den = den_pool.tile([1, 512], F32, tag="den", name="den")
                  nc.scalar.activation(den, cps[64:65, :], AF.Copy)
                  recr = den_pool.tile([1, 512], F32, tag="recr", name="recr")
                  # approx_fast needs SBUF input (garbage straight from PSUM);
                  # ~51 ULP, ~5x faster than the 6-cpe exact reciprocal.
                  nc.vector.reciprocal_approx_fast(recr, den)
                  recb = den_pool.tile([64, 512], F32, tag="recb", name="recb")
                  nc.gpsimd.partition_broadcast(recb, recr)
                  nc.vector.tensor_mul(
                      ctxT_sb[hp:hp + 64, m, qc * 512:(qc + 1) * 512],
                      cps[0:64, :], recb)

              def yproj(tt):
                  ps = ps_pool.tile([P, 2, 512], F32, tag="ps", name="yps")
                  for kc in range(4):
                      lhsT = ctxT_sb[:, kc, tt * P:(tt + 1) * P]
                      nc.tensor.matmul(ps[:, 0, :], lhsT, wp_sb[:, kc, 0:512],
                                       start=(kc == 0), stop=(kc == 3))
                      nc.tensor.matmul(ps[:, 1, :], lhsT, wp_sb[:, kc, 512:],
                                       start=(kc == 0), stop=(kc == 3))
                  ysb = y_pool.tile([P, C], MMD, tag="y", name="ysb")
                  if tt % 2 == 0:
                      nc.scalar.activation(
                          ysb.rearrange("p (a b) -> p a b", a=2), ps, AF.Copy)
                  else:
                      nc.vector.tensor_copy(
                          ysb.rearrange("p (a b) -> p a b", a=2), ps)
                  if phase != 'noy':
                      nc.scalar.dma_start(
                          out=y.ap()[tt * P:(tt + 1) * P, :], in_=ysb)

              def attention(with_yproj=True):
                  segments = [(m, qc) for m in range(4) for qc in (0, 1)]
                  # fill work to cover exp latency: next pair's projection
                  # groups for m<3; yproj for the last pair.
                  fills = {}
                  for m in range(3):
                      fills[(m, 0, 2)] = lambda m=m: qk_proj_group(m + 1, 0)
                      fills[(m, 1, 2)] = lambda m=m: qk_proj_group(m + 1, 1)
                  if with_yproj:
                      for kt in range(2, 6):
                          fills[(3, 1, kt)] = lambda kt=kt: yproj(kt - 2)
                  # ctx runs TWO steps behind scores: exp(kt) gets a full
                  # extra step of engine time before ctx(kt) needs it, so the
                  # in-order PE stream never waits on a just-computed exp.
                  for idx, (m, qc) in enumerate(segments):
                      last = 3 if qc == 0 else 7
                      S_pair(m, qc, 0)
                      S_pair(m, qc, 1)
                      for kt in range(2, last + 1):
                          S_pair(m, qc, kt)
                          if (m, qc, kt) in fills:
                              fills[(m, qc, kt)]()
                          C_pair(m, qc, kt - 2)
                      if idx + 1 < len(segments):
                          nm, nqc = segments[idx + 1]
                          S_pair(nm, nqc, 0)
                      C_pair(m, qc, last - 1)
                      if idx + 1 < len(segments):
                          S_pair(nm, nqc, 1)
                      C_pair(m, qc, last)
                      norm_write(m, 0, qc)
                      norm_write(m, 1, qc)
                  if with_yproj:
                      for tt in range(4, 8):
                          yproj(tt)

              if phase == 'empty':
                  nc.vector.memset(bq_sb, 0.0)
              elif phase == 'dma':
                  for di, sb_t in enumerate((xT_sb, wq_sb, wk_sb, wv_sb, wp_sb)):
                      nch = sb_t.shape[1]
                      nc.sync.dma_start(
                          out=dbg.ap()[:, 0, di * 64:di * 64 + nch * 8],
                          in_=sb_t[:, :, :8])
              elif phase == 'proj':
                  for mc in range(4):
                      qk_proj(mc)
                  nc.sync.dma_start(out=dbg.ap()[:, 0, :4096],
                                    in_=qT_sb.rearrange("p a b c -> p (a b c)"))
                  nc.sync.dma_start(out=dbg.ap()[:, 1, :4096],
                                    in_=kT_sb.rearrange("p a b c -> p (a b c)"))
                  for kt8 in range(8):
                      nc.sync.dma_start(
                          out=dbg.ap()[:, 2, kt8 * 520:kt8 * 520 + 520]
                          .rearrange("p (b c) -> p b c", b=8),
                          in_=v_sb[:, kt8, :, 0:65])
              elif phase == 'attn':
                  qk_proj(0)
                  attention(with_yproj=False)
                  nc.sync.dma_start(out=dbg.ap()[:, 0, :4096],
                                    in_=ctxT_sb.rearrange("p a b -> p (a b)"))
              elif phase in ('a_nodbg', 'noexp', 'nonorm', 'noboth'):
                  qk_proj(0)
                  attention(with_yproj=False)
                  nc.sync.dma_start(out=dbg.ap()[:, 0, :1],
                                    in_=ctxT_sb[:, 0, 0:1])
              else:
                  qk_proj(0)
                  attention(with_yproj=True)
            _ls.close()
    nc.compile()
    return nc


_NC = None


def _get_nc():
    global _NC
    if _NC is None:
        _NC = build_nc()
    return _NC


def make_in_maps(x, Wq, bq, Wk, Wv, Wp, mm_dtype=None):
    """Per-core input dicts."""
    import ml_dtypes
    MMD = mm_dtype or MM_DTYPE
    cvt = ((lambda a: np.ascontiguousarray(a).astype(ml_dtypes.bfloat16))
           if MMD == BF16 else np.ascontiguousarray)
    valid = (np.arange(P)[None, :] >= np.arange(P)[:, None])
    mask01 = valid.astype(np.float32)
    in_maps = []
    for core in range(N_CORES):
        b = core // 2
        g = core % 2
        cs = slice(g * CS, (g + 1) * CS)
        blob = np.concatenate([
            x[b].T.reshape(P, 8 * 1024),
            (Wq[:, cs] * np.float32(0.125)).reshape(P, 8 * 512),
            Wk[:, cs].reshape(P, 8 * 512),
            Wv[:, cs].reshape(P, 8 * 512),
            Wp[cs, :].reshape(4, P, 1024).transpose(1, 0, 2).reshape(P, 4096),
            mask01, mask01,
        ], axis=1)
        assert blob.shape == (P, BLOB_LEN)
        in_maps.append(dict(
            blob=cvt(blob),
            bq=np.ascontiguousarray((bq[cs] * np.float32(0.125))
                                    .reshape(4, P).T),
        ))
    return in_maps


def combine(parts, Wq, bv, Wp, bp):
    """parts: list of 8 per-core partial y arrays -> full [B, T, C] output."""
    out = np.stack([parts[2 * b].astype(np.float32)
                    + parts[2 * b + 1].astype(np.float32)
                    for b in range(B)])
    out += (bv @ Wp + bp)[None, None, :]
    return out.astype(np.float32)


def kernel(**inputs):
    x = np.asarray(inputs["x"], np.float32)
    Wq = np.asarray(inputs["Wq"], np.float32)
    bq = np.asarray(inputs["bq"], np.float32)
    Wk = np.asarray(inputs["Wk"], np.float32)
    Wv = np.asarray(inputs["Wv"], np.float32)
    Wp = np.asarray(inputs["Wp"], np.float32)
    bv = np.asarray(inputs["bv"], np.float32)
    bp = np.asarray(inputs["bp"], np.float32)
    # bk intentionally unused: it shifts every score of a query row by the
    # same amount, which softmax cancels exactly.

    nc = _get_nc()
    in_maps = make_in_maps(x, Wq, bq, Wk, Wv, Wp)
    res = run_bass_kernel_spmd(nc, in_maps, core_ids=list(range(N_CORES)))
    parts = [res.results[c]["y"] for c in range(N_CORES)]
    return combine(parts, Wq, bv, Wp, bp)
